# revision 22
# baseline (speedup 1.0000x reference)
"""Trainium2 Bass kernel for nn_MirrorSystem_24902220382482 (vq_codebook).

Math (per token t, symbol n):
  z = concat(z_real, z_imag)                  [T, 512]
  z_norm = z / (||z|| + 1e-6)
  d_content[t,n] = (||z_norm||^2 + ||c_n||^2 - 2 z_norm.c_n) / 512  (relu never binds)
  batch_var = var(|z_complex|, ddof=1) over ALL elements (global!)
  ew = 0.1 * softplus(batch_var / (1+1e-6))
  gb = prev @ A ; d_total = d_content - ew*sigmoid(gb)
  probs = softmax(-d_total); z_q = probs @ c ; energy = mean(d_total)

Mapping: shard tokens across 8 cores (core c <- batch c), token-major
128-token tiles, fp16 matmuls (fp32 psum). Global variance via on-device
AllReduce. sigmoid via tanh (shares the exp ACT table -> no table thrash):
  logits[t,n] = ew*sig + dot/256 - nsqn/512 - csq/512
  exp(logits) = e1 * e2
    e1 = exp((ew/2)*tanh(gb/2) + (ew/2 - nsqn/512))        [ACT, psum gb]
    e2 = exp((inv/16) * S2), S2 = z16 @ (c.T/16) + (16*nrm) x (-csq/512)
         (rank-1 row folds the per-symbol csq bias; inv*nrm = 1 - O(5e-8))
  z_q = (efT.T @ c) / sum_n(ef);  energy from row sums.
Transposes of z/prev via xbar DMA-transpose from f16 DRAM staging,
group-pipelined (4 tiles per group); exp transpose on the PE.
"""
import sys
import numpy as np

sys.path.insert(0, "/opt/trn_rl_repo")

B, S, DIM, NSYM = 8, 4096, 256, 1024
NCORES = 8
TOK_PER_CORE = B * S // NCORES          # 4096
N_TILES_FULL = TOK_PER_CORE // 128      # 32
EPS = 1e-6
LAMBDA = 0.1
GROUP = 4                               # token-tiles per staging group

TRACE = False
LAST_RESULTS = None

_cache = {}


def build_program(n_tiles=N_TILES_FULL, use_collective=True, ncores=NCORES,
                  nelem_total=None):
    from contextlib import ExitStack
    import concourse.mybir as mybir
    import concourse.tile as tile
    from concourse import bacc
    from concourse.masks import make_identity

    F16 = mybir.dt.float16
    F32 = mybir.dt.float32
    AX = mybir.AxisListType
    AF = mybir.ActivationFunctionType
    OP = mybir.AluOpType

    group = min(GROUP, n_tiles)
    assert n_tiles % group == 0
    n_groups = n_tiles // group
    gtok = group * 128
    T = n_tiles * 128
    if nelem_total is None:
        nelem_total = (T * ncores) * DIM if use_collective else T * DIM
    Nf = float(nelem_total)

    nc = bacc.Bacc("TRN2", target_bir_lowering=False, debug=False,
                   num_devices=ncores)

    zr_d = nc.dram_tensor("zr", [T, DIM], F32, kind="ExternalInput")
    zi_d = nc.dram_tensor("zi", [T, DIM], F32, kind="ExternalInput")
    pv_d = nc.dram_tensor("pv", [T, NSYM], F32, kind="ExternalInput")
    a_d = nc.dram_tensor("a16", [NSYM, NSYM], F16, kind="ExternalInput")
    ct_d = nc.dram_tensor("ct16", [512, NSYM], F16, kind="ExternalInput")   # c.T/16
    cn_d = nc.dram_tensor("cn16", [NSYM, 512], F16, kind="ExternalInput")   # c
    cs_d = nc.dram_tensor("cst16", [512, 1], F16, kind="ExternalInput")     # c_sum/16
    cr_d = nc.dram_tensor("csqr16", [1, NSYM], F16, kind="ExternalInput")   # -csq/512
    cq_d = nc.dram_tensor("csqs", [1, 1], F32, kind="ExternalInput")        # sum(csq)

    zq_d = nc.dram_tensor("zq", [T, 512], F32, kind="ExternalOutput")
    ep_d = nc.dram_tensor("ep", [1, 1], F32, kind="ExternalOutput")

    with tile.TileContext(nc) as tc:
        with ExitStack() as ctx:
            consts = ctx.enter_context(tc.tile_pool(name="consts", bufs=1))
            stats = ctx.enter_context(tc.tile_pool(name="stats", bufs=1))
            wa = ctx.enter_context(tc.tile_pool(name="wa", bufs=3))
            wg = ctx.enter_context(tc.tile_pool(name="wg", bufs=2))
            # zT tiles are produced in phase A but consumed in phase B (after
            # the collective barrier) — they must all be resident, else their
            # slot waits deadlock against the barrier.
            zts = ctx.enter_context(tc.tile_pool(name="zts", bufs=n_groups))
            wb = ctx.enter_context(tc.tile_pool(name="wb", bufs=2))
            ps1 = ctx.enter_context(tc.tile_pool(name="ps1", bufs=1, space="PSUM"))
            ps2 = ctx.enter_context(tc.tile_pool(name="ps2", bufs=2, space="PSUM"))
            dram = ctx.enter_context(tc.tile_pool(name="dram", bufs=2, space="DRAM"))

            # ---------------- constants ----------------
            ident = consts.tile([128, 128], F16)
            make_identity(nc, ident[:])
            ones_col = consts.tile([128, 1], F32)
            nc.vector.memset(ones_col[:], 1.0)
            ones_row = consts.tile([1, 128], F32)
            nc.vector.memset(ones_row[:], 1.0)

            a_sb = consts.tile([128, 8, NSYM], F16)
            nc.sync.dma_start(out=a_sb[:], in_=a_d[:].rearrange("(c p) m -> p c m", p=128))
            ct_sb = consts.tile([128, 4, NSYM], F16)
            nc.sync.dma_start(out=ct_sb[:], in_=ct_d[:].rearrange("(c p) m -> p c m", p=128))
            cn_sb = consts.tile([128, 8, 512], F16)
            nc.sync.dma_start(out=cn_sb[:], in_=cn_d[:].rearrange("(c p) m -> p c m", p=128))
            cs_sb = consts.tile([128, 4, 1], F16)
            nc.sync.dma_start(out=cs_sb[:], in_=cs_d[:].rearrange("(c p) m -> p c m", p=128))
            cr_sb = consts.tile([1, NSYM], F16)
            nc.sync.dma_start(out=cr_sb[:], in_=cr_d[:])
            cq_sb = consts.tile([1, 1], F32)
            nc.sync.dma_start(out=cq_sb[:], in_=cq_d[:])

            # ---------------- persistent stats [128, n_tiles] ----------------
            nsqs = stats.tile([128, n_tiles], F32)
            magsums = stats.tile([128, n_tiles], F32)
            nsqns = stats.tile([128, n_tiles], F32)
            invs = stats.tile([128, n_tiles], F32)
            thsums = stats.tile([128, n_tiles], F32)
            dotsums = stats.tile([128, n_tiles], F32)
            invs16 = stats.tile([128, n_tiles], F32)
            bias2s = stats.tile([128, n_tiles], F32)
            ew_vec = stats.tile([128, 1], F32)
            ewh_vec = stats.tile([128, 1], F32)
            half_vec = stats.tile([128, 1], F32)
            sixt_vec = stats.tile([128, 1], F32)

            # ================ phase A ================
            zT_groups = []
            for g in range(n_groups):
                r0 = g * gtok
                z_g = wa.tile([128, group, 512], F16, tag="z")
                nc.gpsimd.dma_start(
                    out=z_g[:, :, 0:DIM],
                    in_=zr_d[r0:r0 + gtok, :].rearrange("(j p) d -> p j d", p=128))
                nc.gpsimd.dma_start(
                    out=z_g[:, :, DIM:512],
                    in_=zi_d[r0:r0 + gtok, :].rearrange("(j p) d -> p j d", p=128))

                zT_g = zts.tile([128, 4, gtok], F16, tag="zT")
                for j in range(group):
                    t = g * group + j
                    z_t = z_g[:, j, :]
                    sq_t = wa.tile([128, 512], F16, tag="sq")
                    nc.scalar.activation(sq_t[:], z_t, AF.Square,
                                         accum_out=nsqs[:, t:t + 1])
                    msq_t = wa.tile([128, DIM], F16, tag="msq")
                    nc.vector.tensor_add(msq_t[:], sq_t[:, 0:DIM], sq_t[:, DIM:512])
                    mag_t = wa.tile([128, DIM], F16, tag="mag")
                    nc.scalar.activation(mag_t[:], msq_t[:], AF.Sqrt,
                                         accum_out=magsums[:, t:t + 1])
                    # zT chunks via PE transpose (PE is idle in phase A)
                    for k in range(4):
                        ps_tp = ps2.tile([128, 128], F16, tag="tp")
                        nc.tensor.transpose(ps_tp[:], z_t[:, k * 128:(k + 1) * 128],
                                            ident[:])
                        nc.vector.tensor_copy(
                            zT_g[:, k, j * 128:(j + 1) * 128], ps_tp[:])
                zT_groups.append(zT_g)

            # ---- prev staging: per group, cast fp32->f16 rows to DRAM then
            # 8 transposed loads [n-chunk 128, gtok] (xbar) ----
            pvT_groups = []
            for g in range(n_groups):
                p16_g = dram.tile([gtok, NSYM], F16, tag="p16")
                nc.gpsimd.dma_start(out=p16_g[:],
                                    in_=pv_d[g * gtok:(g + 1) * gtok, :])
                pvT_g = wg.tile([128, 8, gtok], F16, tag="pvT")
                for k in range(8):
                    nc.sync.dma_start_transpose(
                        out=pvT_g[:, k, :], in_=p16_g[:, k * 128:(k + 1) * 128])
                pvT_groups.append(pvT_g)

            # batched: inv = 1/(sqrt(nsq)+eps), nsqn = nsq*inv^2, w = 16*nrm
            nrms = stats.tile([128, n_tiles], F32)
            nc.scalar.activation(nrms[:], nsqs[:], AF.Sqrt)
            nrme = stats.tile([128, n_tiles], F32)
            nc.vector.tensor_scalar_add(nrme[:], nrms[:], EPS)
            nc.vector.reciprocal(invs[:], nrme[:])
            qq = stats.tile([128, n_tiles], F32)
            nc.vector.tensor_mul(qq[:], nsqs[:], invs[:])
            nc.vector.tensor_mul(nsqns[:], qq[:], invs[:])
            w16 = stats.tile([128, n_tiles], F16)
            nc.vector.tensor_scalar_mul(w16[:], nrms[:], 16.0)
            ps_w = ps2.tile([n_tiles, 128], F16, tag="tp")
            nc.tensor.transpose(ps_w[:], w16[:], ident[:])
            w_all = stats.tile([n_tiles, 128], F16)
            nc.vector.tensor_copy(w_all[:], ps_w[:])
            # flatten to one partition-0 row via DRAM so each tile's w-slice
            # has base_partition 0 (required for the rank-1 matmul lhsT)
            w_dram = dram.tile([1, n_tiles * 128], F16, tag="wflat")
            nc.sync.dma_start(
                out=w_dram[:].rearrange("o (a b) -> (o a) b", b=128), in_=w_all[:])
            w_flat = stats.tile([1, n_tiles * 128], F16)
            nc.sync.dma_start(out=w_flat[:], in_=w_dram[:])

            # ================ global variance -> ew ================
            smag = stats.tile([128, 1], F32)
            smsq = stats.tile([128, 1], F32)
            nc.vector.reduce_sum(out=smag[:], in_=magsums[:], axis=AX.X)
            nc.vector.reduce_sum(out=smsq[:], in_=nsqs[:], axis=AX.X)
            cc_sb = stats.tile([128, 2], F32)
            nc.vector.tensor_copy(cc_sb[:, 0:1], smag[:])
            nc.vector.tensor_copy(cc_sb[:, 1:2], smsq[:])

            cc_in = dram.tile([128, 2], F32, tag="ccin")
            cc_out = dram.tile([128, 2], F32, tag="ccout")
            nc.gpsimd.dma_start(out=cc_in[:], in_=cc_sb[:])
            if use_collective:
                nc.gpsimd.collective_compute(
                    "AllReduce", OP.add,
                    replica_groups=[list(range(ncores))],
                    ins=[cc_in[:].opt()], outs=[cc_out[:].opt()])
            else:
                nc.gpsimd.dma_start(out=cc_out[:], in_=cc_in[:])
            cc2_sb = stats.tile([128, 2], F32)
            nc.gpsimd.dma_start(out=cc2_sb[:], in_=cc_out[:])

            ps_var = ps1.tile([1, 2], F32, tag="tiny")
            nc.tensor.matmul(ps_var[:, 0:1], cc2_sb[:, 0:1], ones_col[:],
                             start=True, stop=True)
            nc.tensor.matmul(ps_var[:, 1:2], cc2_sb[:, 1:2], ones_col[:],
                             start=True, stop=True)
            vv = stats.tile([1, 2], F32)
            nc.vector.tensor_copy(vv[:], ps_var[:])
            s1sq = stats.tile([1, 1], F32)
            nc.vector.tensor_mul(s1sq[:], vv[:, 0:1], vv[:, 0:1])
            vtmp = stats.tile([1, 1], F32)
            nc.vector.tensor_scalar_mul(vtmp[:], s1sq[:], -1.0 / Nf)
            nc.vector.tensor_add(vtmp[:], vtmp[:], vv[:, 1:2])
            varv = stats.tile([1, 1], F32)
            nc.vector.tensor_scalar_mul(varv[:], vtmp[:], 1.0 / (Nf - 1.0))
            # softplus(x) = ln(1 + e^x): Exp+Ln share natural_log_exp table
            expv = stats.tile([1, 1], F32)
            nc.scalar.activation(expv[:], varv[:], AF.Exp, scale=1.0 / (1.0 + EPS))
            up = stats.tile([1, 1], F32)
            nc.scalar.activation(up[:], expv[:], AF.Ln, bias=1.0)

            ps_ew = ps1.tile([128, 1], F32, tag="tiny")
            nc.tensor.matmul(ps_ew[:], ones_row[:], up[:], start=True, stop=True)
            nc.vector.tensor_scalar_mul(ew_vec[:], ps_ew[:], LAMBDA)
            nc.vector.tensor_scalar_mul(ewh_vec[:], ew_vec[:], 0.5)
            nc.vector.tensor_scalar(out=half_vec[:], in0=ew_vec[:], scalar1=0.0,
                                    scalar2=0.5, op0=OP.mult, op1=OP.add)
            nc.vector.tensor_scalar(out=sixt_vec[:], in0=ew_vec[:], scalar1=0.0,
                                    scalar2=1.0 / 16.0, op0=OP.mult, op1=OP.add)
            nc.vector.tensor_scalar(out=invs16[:], in0=invs[:],
                                    scalar1=sixt_vec[:, 0:1], scalar2=None,
                                    op0=OP.mult)
            nc.vector.tensor_scalar(out=bias2s[:], in0=nsqns[:],
                                    scalar1=-1.0 / 512.0, scalar2=ewh_vec[:, 0:1],
                                    op0=OP.mult, op1=OP.add)

            # ================ phase B ================
            for t in range(n_tiles):
                g, j = divmod(t, group)
                pvT_g = pvT_groups[g]
                zT_g = zT_groups[g]
                jcol = slice(j * 128, (j + 1) * 128)

                # dotsum: z16 @ c_sum/16 -> [128,1]
                ps_ds = ps1.tile([128, 1], F32, tag="tiny")
                for k in range(4):
                    nc.tensor.matmul(ps_ds[:], zT_g[:, k, jcol], cs_sb[:, k, :],
                                     start=(k == 0), stop=(k == 3))
                nc.vector.tensor_copy(dotsums[:, t:t + 1], ps_ds[:])

                # gb = prev @ A
                ps_gb = ps1.tile([128, NSYM], F32, tag="gb")
                for k in range(8):
                    for h in range(2):
                        nc.tensor.matmul(ps_gb[:, h * 512:(h + 1) * 512],
                                         pvT_g[:, k, jcol],
                                         a_sb[:, k, h * 512:(h + 1) * 512],
                                         start=(k == 0), stop=(k == 7))

                th_t = wb.tile([128, NSYM], F16, tag="th")
                nc.scalar.activation(th_t[:], ps_gb[:], AF.Tanh,
                                     scale=0.5,
                                     accum_out=thsums[:, t:t + 1])

                # S2 = z16 @ c.T/16 + (16*nrm) x (-csq/512)
                ps_s2 = ps1.tile([128, NSYM], F32, tag="s2")
                for h in range(2):
                    hs = slice(h * 512, (h + 1) * 512)
                    nc.tensor.matmul(ps_s2[:, hs],
                                     w_flat[:, t * 128:(t + 1) * 128], cr_sb[:, hs],
                                     start=True, stop=False)
                    for k in range(4):
                        nc.tensor.matmul(ps_s2[:, hs], zT_g[:, k, jcol],
                                         ct_sb[:, k, hs],
                                         start=False, stop=(k == 3))

                e1_t = wb.tile([128, NSYM], F16, tag="e1")
                nc.scalar.activation(e1_t[:], th_t[:], AF.Exp,
                                     bias=bias2s[:, t:t + 1], scale=ewh_vec[:, 0:1])
                e2_t = wb.tile([128, NSYM], F16, tag="e2")
                nc.scalar.activation(e2_t[:], ps_s2[:], AF.Exp,
                                     scale=invs16[:, t:t + 1])
                ef_t = wb.tile([128, NSYM], F16, tag="ef")
                nc.vector.tensor_mul(ef_t[:], e1_t[:], e2_t[:])
                se_t = wb.tile([128, 1], F32, tag="se")
                nc.vector.reduce_sum(out=se_t[:], in_=ef_t[:], axis=AX.X)

                efT_t = wb.tile([128, NSYM], F16, tag="efT")
                for k in range(8):
                    ps_tp = ps2.tile([128, 128], F16, tag="tp")
                    nc.tensor.transpose(ps_tp[:], ef_t[:, k * 128:(k + 1) * 128],
                                        ident[:])
                    nc.vector.tensor_copy(efT_t[:, k * 128:(k + 1) * 128], ps_tp[:])

                ps_zq = ps1.tile([128, 512], F32, tag="zq")
                for k in range(8):
                    nc.tensor.matmul(ps_zq[:], efT_t[:, k * 128:(k + 1) * 128],
                                     cn_sb[:, k, :], start=(k == 0), stop=(k == 7))

                seinv_t = wb.tile([128, 1], F32, tag="seinv")
                nc.vector.reciprocal(seinv_t[:], se_t[:])
                zqo_t = wb.tile([128, 512], F32, tag="zqo")
                nc.vector.tensor_scalar(out=zqo_t[:], in0=ps_zq[:],
                                        scalar1=seinv_t[:], scalar2=None,
                                        op0=OP.mult)
                nc.sync.dma_start(out=zq_d[t * 128:(t + 1) * 128, :], in_=zqo_t[:])

            # ================ energy ================
            en1 = stats.tile([128, n_tiles], F32)
            nc.vector.tensor_scalar(out=en1[:], in0=thsums[:],
                                    scalar1=ewh_vec[:, 0:1], scalar2=None,
                                    op0=OP.mult)
            en2 = stats.tile([128, n_tiles], F32)
            nc.vector.tensor_mul(en2[:], dotsums[:], invs16[:])
            nc.vector.tensor_add(en1[:], en1[:], en2[:])
            en3 = stats.tile([128, n_tiles], F32)
            nc.vector.tensor_scalar_mul(en3[:], nsqns[:], -2.0)
            nc.vector.tensor_add(en1[:], en1[:], en3[:])
            erow = stats.tile([128, 1], F32)
            nc.vector.reduce_sum(out=erow[:], in_=en1[:], axis=AX.X)
            ps_e = ps1.tile([1, 1], F32, tag="tiny")
            nc.tensor.matmul(ps_e[:], erow[:], ones_col[:], start=True, stop=True)
            a1 = stats.tile([1, 1], F32)
            nc.vector.tensor_scalar_mul(a1[:], ew_vec[0:1, 0:1], float(T) * 512.0)
            a2 = stats.tile([1, 1], F32)
            nc.vector.tensor_scalar_mul(a2[:], cq_sb[:], -float(T) / 512.0)
            nc.vector.tensor_add(a1[:], a1[:], ps_e[:])
            nc.vector.tensor_add(a1[:], a1[:], a2[:])
            epv = stats.tile([1, 1], F32)
            nc.vector.tensor_scalar_mul(epv[:], a1[:], -1.0)
            nc.sync.dma_start(out=ep_d[:], in_=epv[:])

    nc.compile()
    return nc


def _host_constants(codebook, adjacency_energy):
    c = codebook.astype(np.float64)                      # [1024, 512]
    csq = (c ** 2).sum(1)                                # [1024]
    return {
        "a16": np.ascontiguousarray(adjacency_energy.astype(np.float16)),
        "ct16": np.ascontiguousarray((c.T / 16.0).astype(np.float16)),
        "cn16": np.ascontiguousarray(c.astype(np.float16)),
        "cst16": np.ascontiguousarray((c.sum(0) / 16.0)[:, None].astype(np.float16)),
        "csqr16": np.ascontiguousarray((-csq / 512.0)[None, :].astype(np.float16)),
        "csqs": np.array([[csq.sum()]], dtype=np.float32),
    }


def kernel(**inputs):
    """Full inputs in, full outputs out. Shards batch across 8 NeuronCores."""
    global LAST_RESULTS
    from concourse import bass_utils

    z_real = np.asarray(inputs["z_real"], dtype=np.float32)
    z_imag = np.asarray(inputs["z_imag"], dtype=np.float32)
    prev = np.asarray(inputs["prev_sym_dist"], dtype=np.float32)
    codebook = np.asarray(inputs["codebook"], dtype=np.float32)
    adj = np.asarray(inputs["adjacency_energy"], dtype=np.float32)

    key = ("full", N_TILES_FULL, NCORES)
    if key not in _cache:
        _cache[key] = build_program(N_TILES_FULL, True, NCORES)
    nc = _cache[key]

    consts = _host_constants(codebook, adj)
    in_maps = []
    for c in range(NCORES):
        in_maps.append({
            "zr": np.ascontiguousarray(z_real[c]),
            "zi": np.ascontiguousarray(z_imag[c]),
            "pv": np.ascontiguousarray(prev[c]),
            **consts,
        })

    res = bass_utils.run_bass_kernel_spmd(
        nc, in_maps, core_ids=list(range(NCORES)), trace=TRACE)
    LAST_RESULTS = res

    zq = np.stack([res.results[c]["zq"] for c in range(NCORES)], axis=0)
    ep = sum(float(res.results[c]["ep"][0, 0]) for c in range(NCORES))
    energy = np.float32(ep / (B * S * NSYM))
    return zq.astype(np.float32), energy
